# revision 1
# baseline (speedup 1.0000x reference)
"""MoE FFN (top-1 routing) on 8 Trainium2 NeuronCores.

Strategy
--------
Host router: logits in fp64 -> argmax matches the fp32 reference exactly
(min top-2 logit gap ~2e-4 >> fp32 matmul noise); tokens are grouped by
expert (stable order), so the grouped-by-expert concatenation IS the
reference output order - no inverse permutation needed.

Device (default impl "v2", ff-parallel): every core runs ALL experts but
only its quarter (4 of 32 ff-tiles) of D_FF - the expert dimension splits
along D_FF, which keeps per-core work perfectly balanced regardless of
routing skew (PE streams sum(C_e) columns instead of 8*max(C_e)).
Per-core fp16 partial outputs are summed on the host (the 4-j-tile
contraction is linear). Both layers keep weights stationary in the PE
array with tokens as the moving operand, so no on-chip transpose is
needed: H^T = relu(W1^T X^T + b1), Y^T_partial = W2^T H^T.

Matmuls run in fp16 (1 PE cycle/row, same as bf16, but 10-bit mantissa:
rel err ~5e-4 vs ~3e-3 for bf16 on this unit-scale data) with fp32 PSUM
accumulation. Weights are host-packed into contiguous ~1MB chunks
streamed in compute-need order over both HWDGE trigger lanes (SP=x,
ACT=weights); dummy warm-up matmuls keep the PE HAM clock at 2.4GHz
through the initial DMA wait. L1 runs one expert ahead of L2 so L2's
long-resident inputs absorb any DMA lateness.

KERNEL_IMPL=v1 selects the simpler expert-parallel fallback (core e owns
expert e, tokens padded to max count).
"""

import os
import sys

import numpy as np

sys.path.insert(0, "/opt/trn_rl_repo")

import ml_dtypes  # noqa: E402

D = 1024
E = 8
F = 4096
P = 128
DT = D // P  # 8 d-tiles
FT = F // P  # 32 ff-tiles
MT = D // P  # 8 dout-tiles

BF16 = ml_dtypes.bfloat16

# set by the last kernel() call; test harness reads exec_time_ns from here
last_results = None

_prog_cache = {}


def _ensure_ntff_hook():
    """The agent image's ``antenv`` lacks ``axon_hooks``; install a shim so
    run_bass_kernel_spmd(trace=True) can reach NTFF profiling (degrades to
    no-trace if anything is missing)."""
    try:
        import antenv.axon_hooks  # noqa: F401
        return
    except ImportError:
        pass
    try:
        import types
        import antenv

        mod = types.ModuleType("antenv.axon_hooks")
        _state = {"hook": None}
        mod.set_axon_ntff_profile_hook = lambda h: _state.__setitem__("hook", h)
        mod.get_axon_ntff_profile_hook = lambda: _state["hook"]
        sys.modules["antenv.axon_hooks"] = mod
        antenv.axon_hooks = mod
        try:
            from trn_agent_boot.trn_boot import _ntff_profile_via_ctypes

            mod.set_axon_ntff_profile_hook(
                _ntff_profile_via_ctypes("/opt/axon/libaxon_pjrt.so")
            )
        except Exception:
            pass
    except Exception:
        pass


def _tok_tiles(C):
    """Split C tokens into moving-operand tiles of <=512 (PSUM bank limit)."""
    tiles = []
    t0 = 0
    while t0 < C:
        tn = min(512, C - t0)
        tiles.append((t0, tn))
        t0 += tn
    return tiles


def _build(C, compute_dt_name):
    import concourse.mybir as mybir
    from concourse import bacc
    from concourse.tile import TileContext

    cdt = getattr(mybir.dt, compute_dt_name)
    f32 = mybir.dt.float32
    AF = mybir.ActivationFunctionType

    tok = _tok_tiles(C)
    nc = bacc.Bacc(
        "TRN2",
        target_bir_lowering=False,
        debug=False,
        enable_asserts=False,
        num_devices=E,
    )

    xt_d = nc.declare_dram_parameter("xt", [P, DT * C], cdt, isOutput=False)
    w1_d = nc.declare_dram_parameter("w1", [DT, P, 4 * DT * P], cdt, isOutput=False)
    w2_d = nc.declare_dram_parameter("w2", [MT, P, FT * P], cdt, isOutput=False)
    b1_d = nc.declare_dram_parameter("b1", [P, FT], f32, isOutput=False)
    b2_d = nc.declare_dram_parameter("b2", [P, MT], f32, isOutput=False)
    yt_d = nc.declare_dram_parameter("yt", [MT, P, C], f32, isOutput=True)

    with TileContext(nc) as tc:
        with (
            tc.tile_pool(name="const", bufs=1) as constp,
            tc.tile_pool(name="xp", bufs=1) as xp,
            tc.tile_pool(name="w1p", bufs=3) as w1p,
            tc.tile_pool(name="w2p", bufs=3) as w2p,
            tc.tile_pool(name="hp", bufs=1) as hp,
            tc.tile_pool(name="yp", bufs=2) as yp,
            tc.tile_pool(name="ps1", space="PSUM", bufs=2) as ps1,
            tc.tile_pool(name="ps2", space="PSUM", bufs=2) as ps2,
        ):
            x_sb = xp.tile([P, DT * C], cdt, tag="x", name="x_sb")
            nc.sync.dma_start(x_sb[:], xt_d[:])
            b1_sb = constp.tile([P, FT], f32, tag="b1", name="b1_sb")
            nc.sync.dma_start(b1_sb[:], b1_d[:])
            b2_sb = constp.tile([P, MT], f32, tag="b2", name="b2_sb")
            nc.sync.dma_start(b2_sb[:], b2_d[:])

            h_tiles = [
                hp.tile([P, C], cdt, tag=f"h{j}", name=f"h{j}") for j in range(FT)
            ]

            # ---- layer 1: H^T[j] = relu(W1^T X^T + b1), j = ff tile ----
            for jb in range(DT):  # 8 chunks of 4 ff-tiles (1MB each)
                w1_sb = w1p.tile([P, 4 * DT * P], cdt, tag="w1c", bufs=3,
                                 name=f"w1c{jb}")
                nc.sync.dma_start(w1_sb[:], w1_d[jb])
                for jj in range(4):
                    j = jb * 4 + jj
                    pss = [
                        ps1.tile([P, tn], f32, tag=f"psA{ti}", bufs=2,
                                 name=f"ps_{j}_{ti}")
                        for ti, (t0, tn) in enumerate(tok)
                    ]
                    for i in range(DT):
                        lhsT = w1_sb[:, (jj * DT + i) * P:(jj * DT + i + 1) * P]
                        for ti, (t0, tn) in enumerate(tok):
                            nc.tensor.matmul(
                                pss[ti][:],
                                lhsT,
                                x_sb[:, i * C + t0:i * C + t0 + tn],
                                start=(i == 0),
                                stop=(i == DT - 1),
                            )
                    for ti, (t0, tn) in enumerate(tok):
                        nc.scalar.activation(
                            h_tiles[j][:, t0:t0 + tn],
                            pss[ti][:],
                            AF.Relu,
                            bias=b1_sb[:, j:j + 1],
                        )

            # ---- layer 2: Y^T[m] = W2^T H^T + b2, m = dout tile ----
            for m in range(MT):
                w2_sb = w2p.tile([P, FT * P], cdt, tag="w2c", bufs=3,
                                 name=f"w2c{m}")
                nc.sync.dma_start(w2_sb[:], w2_d[m])
                y_sb = yp.tile([P, C], f32, tag="y", bufs=2, name=f"y{m}")
                pss = [
                    ps2.tile([P, tn], f32, tag=f"psB{ti}", bufs=2,
                             name=f"psy_{m}_{ti}")
                    for ti, (t0, tn) in enumerate(tok)
                ]
                for j in range(FT):
                    lhsT = w2_sb[:, j * P:(j + 1) * P]
                    for ti, (t0, tn) in enumerate(tok):
                        nc.tensor.matmul(
                            pss[ti][:],
                            lhsT,
                            h_tiles[j][:, t0:t0 + tn],
                            start=(j == 0),
                            stop=(j == FT - 1),
                        )
                for ti, (t0, tn) in enumerate(tok):
                    nc.scalar.activation(
                        y_sb[:, t0:t0 + tn],
                        pss[ti][:],
                        AF.Identity,
                        bias=b2_sb[:, m:m + 1],
                    )
                nc.sync.dma_start(yt_d[m], y_sb[:])

    nc.compile()
    return nc


_BUILDER_SRC = r'''"""Device-program builder for the MoE kernel.

This file is written by kernel.py to a content-addressed path under /tmp
and imported from there, so the Bass-captured debug info (source path,
line numbers) -- and therefore the generated BIR bytes and the neuronx
compile-cache key -- are identical no matter where kernel.py itself
lives.
"""

import sys

if "/opt/trn_rl_repo" not in sys.path:
    sys.path.insert(0, "/opt/trn_rl_repo")

D = 1024
E = 8
F = 4096
P = 128
DT = D // P
FT = F // P
MT = D // P


def _tok_tiles(C):
    tiles = []
    t0 = 0
    while t0 < C:
        tn = min(512, C - t0)
        tiles.append((t0, tn))
        t0 += tn
    return tiles


def build_v2(ces):
    """ff-parallel: every core runs ALL experts, but only 4 of the 32 ff
    tiles (its quarter of D_FF, baked into its weight data by the host).
    Partial outputs (fp16) are summed on the host. PE streams exactly
    sum(C_e) columns per (i|m)-tile instead of 8*max(C_e)."""
    import concourse.mybir as mybir
    from concourse import bacc
    from concourse.tile import TileContext

    cdt = mybir.dt.float16
    f32 = mybir.dt.float32
    f16 = mybir.dt.float16
    AF = mybir.ActivationFunctionType

    CT = sum(ces)
    xbase = [sum(ces[:e]) for e in range(E)]
    toks = [_tok_tiles(c) for c in ces]

    nc = bacc.Bacc(
        "TRN2",
        target_bir_lowering=False,
        debug=False,
        enable_asserts=False,
        num_devices=E,
    )

    xt_d = nc.declare_dram_parameter("xt", [P, DT * CT], cdt, isOutput=False)
    w1_d = nc.declare_dram_parameter("w1", [E, P, 4 * DT * P], cdt, isOutput=False)
    w2_d = nc.declare_dram_parameter("w2", [E, P, 4 * MT * P], cdt, isOutput=False)
    b1_d = nc.declare_dram_parameter("b1", [P, E * 4], f32, isOutput=False)
    y_ds = [
        nc.declare_dram_parameter(f"y{e}", [P, MT * ces[e]], f16, isOutput=True)
        for e in range(E)
    ]

    with TileContext(nc) as tc:
        with (
            tc.tile_pool(name="const", bufs=1) as constp,
            tc.tile_pool(name="xp", bufs=1) as xp,
            tc.tile_pool(name="w1p", bufs=4) as w1p,
            tc.tile_pool(name="w2p", bufs=4) as w2p,
            tc.tile_pool(name="hp", bufs=2) as hp,
            tc.tile_pool(name="yp", bufs=2) as yp,
            tc.tile_pool(name="ps1", space="PSUM", bufs=2) as ps1,
            tc.tile_pool(name="ps2", space="PSUM", bufs=2) as ps2,
        ):
            x_sb = xp.tile([P, DT * CT], cdt, tag="x", name="x_sb")
            w1_sbs = {}
            h_all = {}

            def dma_x(e, nsplit, first_engine=None):
                xb = DT * xbase[e]
                n = DT * ces[e]
                step = (n + nsplit - 1) // nsplit
                for s in range(0, n, step):
                    w = min(step, n - s)
                    eng = first_engine if (s == 0 and first_engine) else nc.sync
                    eng.dma_start(
                        x_sb[:, xb + s:xb + s + w], xt_d[:, xb + s:xb + s + w]
                    )

            def dma_w1(e, nsplit=1, first_engine=None):
                w1_sb = w1p.tile([P, 4 * DT * P], cdt, tag="w1c", name=f"w1c{e}")
                w1_sbs[e] = w1_sb
                step = 4 * DT * P // nsplit
                for s in range(0, 4 * DT * P, step):
                    eng = first_engine if (s == 0 and first_engine) else nc.scalar
                    eng.dma_start(
                        w1_sb[:, s:s + step], w1_d[e, :, s:s + step]
                    )

            def emit_l1(e):
                Ce = ces[e]
                xb = DT * xbase[e]
                tok = toks[e]
                w1_sb = w1_sbs[e]
                h_all[e] = []
                for jj in range(4):
                    h = hp.tile([P, Ce], cdt, tag=f"h{jj}", bufs=3,
                                name=f"h{e}_{jj}")
                    h_all[e].append(h)
                    pss = [
                        ps1.tile([P, tn], f32, tag=f"psA{ti}",
                                 name=f"ps_{e}_{jj}_{ti}")
                        for ti, (t0, tn) in enumerate(tok)
                    ]
                    for i in range(DT):
                        lhsT = w1_sb[:, (jj * DT + i) * P:(jj * DT + i + 1) * P]
                        for ti, (t0, tn) in enumerate(tok):
                            nc.tensor.matmul(
                                pss[ti][:],
                                lhsT,
                                x_sb[:, xb + i * Ce + t0:xb + i * Ce + t0 + tn],
                                start=(i == 0),
                                stop=(i == DT - 1),
                            )
                    for ti, (t0, tn) in enumerate(tok):
                        nc.scalar.activation(
                            h[:, t0:t0 + tn],
                            pss[ti][:],
                            AF.Relu,
                            bias=b1_sb[:, e * 4 + jj:e * 4 + jj + 1],
                        )

            w2_sbs = {}

            def dma_w2(e):
                w2_sb = w2p.tile([P, 4 * MT * P], cdt, tag="w2c", name=f"w2c{e}")
                w2_sbs[e] = w2_sb
                nc.scalar.dma_start(w2_sb[:], w2_d[e])

            def emit_l2(e):
                Ce = ces[e]
                tok = toks[e]
                w2_sb = w2_sbs.pop(e)
                y_sb = yp.tile([P, MT * Ce], f16, tag="y", name=f"y{e}")
                for m in range(MT):
                    pss = [
                        ps2.tile([P, tn], f32, tag=f"psB{ti}",
                                 name=f"psy_{e}_{m}_{ti}")
                        for ti, (t0, tn) in enumerate(tok)
                    ]
                    for jj in range(4):
                        lhsT = w2_sb[:, (jj * MT + m) * P:(jj * MT + m + 1) * P]
                        for ti, (t0, tn) in enumerate(tok):
                            nc.tensor.matmul(
                                pss[ti][:],
                                lhsT,
                                h_all[e][jj][:, t0:t0 + tn],
                                start=(jj == 0),
                                stop=(jj == 3),
                            )
                    for ti, (t0, tn) in enumerate(tok):
                        nc.vector.tensor_copy(
                            y_sb[:, m * Ce + t0:m * Ce + t0 + tn],
                            pss[ti][:],
                        )
                    if e == E - 1:
                        nc.sync.dma_start(
                            y_ds[e][:, m * Ce:(m + 1) * Ce],
                            y_sb[:, m * Ce:(m + 1) * Ce],
                        )
                if e != E - 1:
                    nc.sync.dma_start(y_ds[e][:], y_sb[:])
                del h_all[e]

            # startup: x0 per-i on SP lane, w1c0 per-jj on ACT lane, so the
            # first matmul starts as soon as x0_i0 + w1c0_jj0 land.
            dma_x(0, 4)
            dma_w1(0, nsplit=4)
            b1_sb = constp.tile([P, E * 4], f32, tag="b1", name="b1_sb")
            nc.scalar.dma_start(b1_sb[:], b1_d[:])

            # PE warm-up: dummy matmuls on (uninitialized) scratch while the
            # first input DMAs are in flight, so HAM is at K=8/8 when real
            # work starts. The psum result is never read.
            warm_w = constp.tile([P, P], cdt, tag="warmw", name="warm_w")
            warm_x = constp.tile([P, 256], cdt, tag="warmx", name="warm_x")
            nc.vector.memset(warm_w[:], 0.0)
            nc.vector.memset(warm_x[:], 0.0)
            warm_ps = ps2.tile([P, 256], f32, tag="psB0", name="warm_ps")
            for w in range(40):
                nc.tensor.matmul(
                    warm_ps[:], warm_w[:], warm_x[:],
                    start=(w == 0), stop=(w == 39),
                )
            # L1 runs one expert ahead of L2: L2(e-1) is ready-to-run PE work
            # that absorbs any DMA lateness in L1(e)'s inputs.
            emit_l1(0)
            dma_x(1, 2)
            dma_w1(1)
            dma_w2(0)
            emit_l1(1)
            for e in range(2, E):
                emit_l2(e - 2)
                dma_x(e, 2)
                dma_w1(e)
                dma_w2(e - 1)
                emit_l1(e)
            dma_w2(E - 1)
            emit_l2(E - 2)
            emit_l2(E - 1)

    nc.compile()
    return nc


def build_v2_into(ces, out):
    # thread entrypoint: keeps caller frames (kernel.py, driver) out of the
    # Bass-captured tracebacks so the BIR bytes are fully location-independent
    try:
        out["nc"] = build_v2(ces)
    except BaseException as exc:  # noqa: BLE001
        out["exc"] = exc
'''


def _build_v2(ces):
    """Build via a content-addressed module under /tmp so the generated BIR
    (and hence the neuron compile-cache key) is independent of where this
    file lives."""
    import hashlib
    import importlib.util

    h = hashlib.md5(_BUILDER_SRC.encode()).hexdigest()[:12]
    modname = f"_moe_builder_{h}"
    if modname not in sys.modules:
        path = f"/tmp/_moe_builder_{h}.py"
        try:
            if not (os.path.exists(path)
                    and open(path).read() == _BUILDER_SRC):
                tmp = f"{path}.{os.getpid()}.tmp"
                with open(tmp, "w") as f:
                    f.write(_BUILDER_SRC)
                os.replace(tmp, path)
        except OSError:
            import tempfile

            path = os.path.join(tempfile.mkdtemp(), f"{modname}.py")
            with open(path, "w") as f:
                f.write(_BUILDER_SRC)
        spec = importlib.util.spec_from_file_location(modname, path)
        mod = importlib.util.module_from_spec(spec)
        sys.modules[modname] = mod
        spec.loader.exec_module(mod)
    import threading

    out = {}
    t = threading.Thread(
        target=sys.modules[modname].build_v2_into, args=(ces, out)
    )
    t.start()
    t.join()
    if "exc" in out:
        raise out["exc"]
    return out["nc"]


def _run_with_retry(run_fn, nc, in_maps, tmpdir, attempts=4):
    """Transient NRT/device errors (e.g. NRT_EXEC_UNIT_UNRECOVERABLE right
    after another process released the cores) have been observed; retry with
    growing backoff, resetting the jax backend in between (the failed PJRT
    client state does not recover on its own)."""
    import time

    last_exc = None
    for a in range(attempts):
        try:
            return run_fn(nc, in_maps, core_ids=list(range(E)), tmpdir=tmpdir)
        except Exception as exc:  # noqa: BLE001
            last_exc = exc
            time.sleep(5.0 * (a + 1))
            try:
                import jax

                jax.clear_backends()
            except Exception:
                pass
    raise last_exc


def kernel(x, Wg, bg, W1, b1, W2, b2, k):
    global last_results
    _ensure_ntff_hook()
    from concourse.bass_utils import run_bass_kernel_spmd

    compute_dt = os.environ.get("KERNEL_COMPUTE_DT", "bfloat16")
    np_cdt = BF16 if compute_dt == "bfloat16" else np.float32

    impl = os.environ.get("KERNEL_IMPL", "v2")

    x = np.asarray(x)
    B, S, _ = x.shape
    N = B * S
    x_flat = np.ascontiguousarray(x.reshape(N, D)).astype(np.float32)

    # ---- host router (exact vs fp32 reference; see module docstring) ----
    logits = x_flat.astype(np.float64) @ np.asarray(Wg).astype(np.float64)
    logits += np.asarray(bg).astype(np.float64)
    assign = np.argmax(logits, axis=-1)

    idx_per_e = [np.flatnonzero(assign == e) for e in range(E)]
    counts = [len(ix) for ix in idx_per_e]

    W1 = np.asarray(W1, dtype=np.float32)
    W2 = np.asarray(W2, dtype=np.float32)
    b1 = np.asarray(b1, dtype=np.float32)
    b2 = np.asarray(b2, dtype=np.float32)

    tmpdir = os.environ.get("KERNEL_TMPDIR")

    if impl == "v2":
        # slot order: largest expert first (more PE work early to cover the
        # DMA supply ramp; smallest expert last shortens the drain tail)
        perm = list(np.argsort([-c for c in counts], kind="stable"))
        ces = [max(8, (counts[p] + 7) // 8 * 8) for p in perm]
        CT = sum(ces)
        xbase = [sum(ces[:s]) for s in range(E)]

        # shared x: per-slot blocks of [P, DT*Ce]
        xt = np.zeros((P, DT * CT), np.float32)
        for s in range(E):
            e = perm[s]
            xp_ = np.zeros((ces[s], D), np.float32)
            xp_[:counts[e]] = x_flat[idx_per_e[e]]
            xt[:, DT * xbase[s]:DT * xbase[s] + DT * ces[s]] = (
                xp_.T.reshape(DT, P, ces[s]).transpose(1, 0, 2)
                .reshape(P, DT * ces[s])
            )
        xt = np.ascontiguousarray(xt).astype(np.float16)

        W1r = W1[perm].reshape(E, DT, P, FT, P)
        W2r = W2[perm].reshape(E, FT, P, MT, P)
        b1r = b1[perm].reshape(E, FT, P)
        in_maps = []
        for kcore in range(E):
            js = slice(4 * kcore, 4 * kcore + 4)
            w1c = np.ascontiguousarray(
                W1r[:, :, :, js, :].transpose(0, 2, 3, 1, 4)
                .reshape(E, P, 4 * DT * P)
            ).astype(np.float16)
            w2c = np.ascontiguousarray(
                W2r[:, js, :, :, :].transpose(0, 2, 1, 3, 4)
                .reshape(E, P, 4 * MT * P)
            ).astype(np.float16)
            b1c = np.ascontiguousarray(
                b1r[:, js, :].transpose(2, 0, 1).reshape(P, E * 4)
            )
            in_maps.append({"xt": xt, "w1": w1c, "w2": w2c, "b1": b1c})

        key = ("v2", tuple(ces))
        if key not in _prog_cache:
            _prog_cache[key] = _build_v2(ces)
        nc = _prog_cache[key]

        last_results = _run_with_retry(
            run_bass_kernel_spmd, nc, in_maps, tmpdir
        )

        inv = [0] * E
        for s, p in enumerate(perm):
            inv[p] = s
        out = np.empty((N, D), np.float32)
        pos = 0
        for e in range(E):
            s = inv[e]  # slot holding expert e
            cnt = counts[e]
            acc = np.zeros((P, MT, ces[s]), np.float32)
            for kcore in range(E):
                acc += last_results.results[kcore][f"y{s}"].reshape(
                    P, MT, ces[s]
                )
            # acc[p, m, t] -> Y^T[(m p), t] -> rows
            ye = acc.transpose(1, 0, 2).reshape(D, ces[s]).T[:cnt]
            out[pos:pos + cnt] = ye + b2[e]
            pos += cnt
        return out.reshape(B, S, D)

    # ---- v1: expert-parallel, core e owns expert e ----
    C = max(counts)
    C = (C + 7) // 8 * 8  # small alignment pad

    in_maps = []
    for e in range(E):
        cnt = counts[e]
        xp_ = np.zeros((C, D), np.float32)
        xp_[:cnt] = x_flat[idx_per_e[e]]
        # xt[p, i*C + t] = x[t, i*128 + p]
        xt = np.ascontiguousarray(
            xp_.T.reshape(DT, P, C).transpose(1, 0, 2).reshape(P, DT * C)
        ).astype(np_cdt)
        # w1[jb, p, (jj, i, c)] = W1[e][i*128+p, (jb*4+jj)*128+c]
        w1 = np.ascontiguousarray(
            W1[e].reshape(DT, P, DT, 4, P).transpose(2, 1, 3, 0, 4)
            .reshape(DT, P, 4 * DT * P)
        ).astype(np_cdt)
        # w2[m, p, (j, c)] = W2[e][j*128+p, m*128+c]
        w2 = np.ascontiguousarray(
            W2[e].reshape(FT, P, MT, P).transpose(2, 1, 0, 3)
            .reshape(MT, P, FT * P)
        ).astype(np_cdt)
        b1p = np.ascontiguousarray(b1[e].reshape(FT, P).T)
        b2p = np.ascontiguousarray(b2[e].reshape(MT, P).T)
        in_maps.append({"xt": xt, "w1": w1, "w2": w2, "b1": b1p, "b2": b2p})

    key = (C, compute_dt)
    if key not in _prog_cache:
        _prog_cache[key] = _build(C, compute_dt)
    nc = _prog_cache[key]

    last_results = _run_with_retry(
        run_bass_kernel_spmd, nc, in_maps, tmpdir
    )

    # ---- gather: grouped-by-expert concat is exactly the reference order ----
    out = np.empty((N, D), np.float32)
    pos = 0
    for e in range(E):
        cnt = counts[e]
        yt = last_results.results[e]["yt"]  # [MT, P, C] == Y^T [1024, C]
        out[pos:pos + cnt] = yt.reshape(D, C).T[:cnt]
        pos += cnt
    return out.reshape(B, S, D)



# revision 2
# speedup vs baseline: 1.0910x; 1.0910x over previous
"""MoE FFN (top-1 routing) on 8 Trainium2 NeuronCores.

Strategy ("v3", quad-split expert/ff-parallel)
---------------------------------------------
Host router: logits in fp64 -> argmax matches the fp32 reference exactly
(min top-2 logit gap >> fp32 matmul noise); tokens are grouped by expert
(stable order), so the grouped-by-expert concatenation IS the reference
output order - no inverse permutation needed.

Device: experts are sorted by token count and split into 2 groups of 4
(group A = ranks 0,2,4,6; B = ranks 1,3,5,7). Cores 0-3 serve group A,
cores 4-7 group B; core q of a group holds the q-th quarter of D_FF for
all 4 of its experts, so per-core weight traffic stays at the 16.8 MB
(fp16) minimum while x/y traffic drops 2x vs an 8-way ff split (x is
sent only to the 4 cores of the owning group). Slot shapes are padded
rank-wise across the two groups so one SPMD program serves all cores
(pad cost ~1.7%). Per-core partial outputs (fp16, one per F-quarter)
are summed on the host - the F contraction is linear.

This keeps sustained DMA demand ~230 GB/s, well under the ~358 GB/s
per-core HBM limit that the previous 8-way version saturated (which
stalled the PE mid-kernel and re-throttled its HAM clock gate).

Matmuls run in fp16 (1 PE cycle/row, 10-bit mantissa: rel err ~5e-4)
with fp32 PSUM accumulation. Weights stream on the SP HWDGE ring
(nothing else queued there), x/b1/y on the ACT ring. A few dummy
warm-up matmuls keep the PE HAM clock busy through the initial DMA
wait. L1 runs one slot ahead of L2 so L2's long-resident inputs absorb
any DMA lateness.
"""

import os
import sys

import numpy as np

sys.path.insert(0, "/opt/trn_rl_repo")

import ml_dtypes  # noqa: E402

D = 1024
E = 8
F = 4096
P = 128
DT = D // P   # 8 d-tiles (L1 contraction / L2 output)
FT = F // P   # 32 ff-tiles total
NG = 2        # expert groups
GS = E // NG  # experts per group = cores per group = 4
FQ = FT // GS  # ff-tiles per core per expert = 8

BF16 = ml_dtypes.bfloat16

# set by the last kernel() call; test harness reads exec_time_ns from here
last_results = None

_prog_cache = {}


def _ensure_ntff_hook():
    """The agent image's ``antenv`` lacks ``axon_hooks``; install a shim so
    run_bass_kernel_spmd(trace=True) can reach NTFF profiling (degrades to
    no-trace if anything is missing)."""
    try:
        import antenv.axon_hooks  # noqa: F401
        return
    except ImportError:
        pass
    try:
        import types
        import antenv

        mod = types.ModuleType("antenv.axon_hooks")
        _state = {"hook": None}
        mod.set_axon_ntff_profile_hook = lambda h: _state.__setitem__("hook", h)
        mod.get_axon_ntff_profile_hook = lambda: _state["hook"]
        sys.modules["antenv.axon_hooks"] = mod
        antenv.axon_hooks = mod
        try:
            from trn_agent_boot.trn_boot import _ntff_profile_via_ctypes

            mod.set_axon_ntff_profile_hook(
                _ntff_profile_via_ctypes("/opt/axon/libaxon_pjrt.so")
            )
        except Exception:
            pass
    except Exception:
        pass


_BUILDER_SRC = r'''"""Device-program builder for the MoE kernel (v3 quad-split).

This file is written by kernel.py to a content-addressed path under /tmp
and imported from there, so the Bass-captured debug info (source path,
line numbers) -- and therefore the generated BIR bytes and the neuronx
compile-cache key -- are identical no matter where kernel.py itself
lives.
"""

import sys

if "/opt/trn_rl_repo" not in sys.path:
    sys.path.insert(0, "/opt/trn_rl_repo")

P = 128
GS = 4   # expert slots per core
FQ = 8   # local ff-tiles per slot (F/4 = 1024)
DT = 8   # L1 contraction tiles / L2 output tiles


def _tok_tiles(C):
    tiles = []
    t0 = 0
    while t0 < C:
        tn = min(512, C - t0)
        tiles.append((t0, tn))
        t0 += tn
    return tiles


def build_v3(cps):
    """Quad-split: this core holds FQ ff-tiles (a quarter of D_FF) of GS=4
    experts. cps = padded token count per slot (shared across cores)."""
    import concourse.mybir as mybir
    from concourse import bacc
    from concourse.tile import TileContext

    cdt = mybir.dt.float16
    f32 = mybir.dt.float32
    f16 = mybir.dt.float16
    AF = mybir.ActivationFunctionType

    CT = sum(cps)
    xbase = [sum(cps[:s]) for s in range(GS)]
    toks = [_tok_tiles(c) for c in cps]

    nc = bacc.Bacc(
        "TRN2",
        target_bir_lowering=False,
        debug=False,
        enable_asserts=False,
        num_devices=8,
    )

    xt_d = nc.declare_dram_parameter("xt", [P, DT * CT], cdt, isOutput=False)
    w1_d = nc.declare_dram_parameter(
        "w1", [GS, P, FQ * DT * P], cdt, isOutput=False
    )
    w2_d = nc.declare_dram_parameter(
        "w2", [GS, P, DT * FQ * P], cdt, isOutput=False
    )
    b1_d = nc.declare_dram_parameter("b1", [P, GS * FQ], f32, isOutput=False)
    y_ds = [
        nc.declare_dram_parameter(
            f"y{s}", [P, DT * cps[s]], f16, isOutput=True
        )
        for s in range(GS)
    ]

    with TileContext(nc) as tc:
        with (
            tc.tile_pool(name="const", bufs=1) as constp,
            tc.tile_pool(name="xp", bufs=1) as xp,
            tc.tile_pool(name="w1p", bufs=3) as w1p,
            tc.tile_pool(name="w2p", bufs=3) as w2p,
            tc.tile_pool(name="hp", bufs=2) as hp,
            tc.tile_pool(name="yp", bufs=2) as yp,
            tc.tile_pool(name="ps1", space="PSUM", bufs=2) as ps1,
            tc.tile_pool(name="ps2", space="PSUM", bufs=2) as ps2,
        ):
            x_sb = xp.tile([P, DT * CT], cdt, tag="x", name="x_sb")
            w1_sbs = {}
            w2_sbs = {}
            h_sbs = {}

            def dma_x(s, nsplit=1):
                xb = DT * xbase[s]
                n = DT * cps[s]
                step = (n + nsplit - 1) // nsplit
                for c0 in range(0, n, step):
                    w = min(step, n - c0)
                    nc.scalar.dma_start(
                        x_sb[:, xb + c0:xb + c0 + w],
                        xt_d[:, xb + c0:xb + c0 + w],
                    )

            def dma_w1(s, nsplit=1):
                w1_sb = w1p.tile([P, FQ * DT * P], cdt, tag="w1c",
                                 name=f"w1c{s}")
                w1_sbs[s] = w1_sb
                step = FQ * DT * P // nsplit
                for c0 in range(0, FQ * DT * P, step):
                    nc.sync.dma_start(
                        w1_sb[:, c0:c0 + step], w1_d[s, :, c0:c0 + step]
                    )

            def dma_w2(s):
                w2_sb = w2p.tile([P, DT * FQ * P], cdt, tag="w2c",
                                 name=f"w2c{s}")
                w2_sbs[s] = w2_sb
                nc.sync.dma_start(w2_sb[:], w2_d[s])

            def emit_l1(s):
                Cs = cps[s]
                xb = DT * xbase[s]
                tok = toks[s]
                w1_sb = w1_sbs[s]
                h_sb = hp.tile([P, FQ * Cs], cdt, tag="h", name=f"h{s}")
                h_sbs[s] = h_sb
                for jj in range(FQ):
                    pss = [
                        ps1.tile([P, tn], f32, tag=f"psA{ti}",
                                 name=f"ps_{s}_{jj}_{ti}")
                        for ti, (t0, tn) in enumerate(tok)
                    ]
                    for i in range(DT):
                        lhsT = w1_sb[:, (jj * DT + i) * P:(jj * DT + i + 1) * P]
                        for ti, (t0, tn) in enumerate(tok):
                            nc.tensor.matmul(
                                pss[ti][:],
                                lhsT,
                                x_sb[:, xb + i * Cs + t0:xb + i * Cs + t0 + tn],
                                start=(i == 0),
                                stop=(i == DT - 1),
                            )
                    for ti, (t0, tn) in enumerate(tok):
                        nc.scalar.activation(
                            h_sb[:, jj * Cs + t0:jj * Cs + t0 + tn],
                            pss[ti][:],
                            AF.Relu,
                            bias=b1_sb[:, s * FQ + jj:s * FQ + jj + 1],
                        )

            def emit_l2(s):
                Cs = cps[s]
                tok = toks[s]
                w2_sb = w2_sbs.pop(s)
                h_sb = h_sbs.pop(s)
                y_sb = yp.tile([P, DT * Cs], f16, tag="y", name=f"y{s}")
                last = (s == GS - 1)
                for m in range(DT):
                    pss = [
                        ps2.tile([P, tn], f32, tag=f"psB{ti}",
                                 name=f"psy_{s}_{m}_{ti}")
                        for ti, (t0, tn) in enumerate(tok)
                    ]
                    for j in range(FQ):
                        lhsT = w2_sb[:, (m * FQ + j) * P:(m * FQ + j + 1) * P]
                        for ti, (t0, tn) in enumerate(tok):
                            nc.tensor.matmul(
                                pss[ti][:],
                                lhsT,
                                h_sb[:, j * Cs + t0:j * Cs + t0 + tn],
                                start=(j == 0),
                                stop=(j == FQ - 1),
                            )
                    for ti, (t0, tn) in enumerate(tok):
                        nc.vector.tensor_copy(
                            y_sb[:, m * Cs + t0:m * Cs + t0 + tn],
                            pss[ti][:],
                        )
                    if last:
                        nc.scalar.dma_start(
                            y_ds[s][:, m * Cs:(m + 1) * Cs],
                            y_sb[:, m * Cs:(m + 1) * Cs],
                        )
                if not last:
                    nc.scalar.dma_start(y_ds[s][:], y_sb[:])

            # startup: x0 per-2-i-tiles on ACT lane, w1c0 per-2-jj on SP
            # lane, so the first matmul starts as soon as the first chunk
            # of each lands.
            dma_x(0, nsplit=4)
            b1_sb = constp.tile([P, GS * FQ], f32, tag="b1", name="b1_sb")
            nc.scalar.dma_start(b1_sb[:], b1_d[:])
            dma_w1(0, nsplit=4)
            dma_x(1)
            dma_w1(1)

            # PE warm-up: dummy matmuls on memset scratch while the first
            # input DMAs are in flight, so the HAM clock-gate window opens
            # early. The psum result is never read. Short enough (~1.7us
            # cold) not to delay the first real matmul.
            warm_w = constp.tile([P, P], cdt, tag="warmw", name="warm_w")
            warm_x = constp.tile([P, 256], cdt, tag="warmx", name="warm_x")
            nc.vector.memset(warm_w[:], 0.0)
            nc.vector.memset(warm_x[:], 0.0)
            warm_ps = ps2.tile([P, 256], f32, tag="psB1", name="warm_ps")
            for w in range(8):
                nc.tensor.matmul(
                    warm_ps[:], warm_w[:], warm_x[:],
                    start=(w == 0), stop=(w == 7),
                )
            # L1 runs one slot ahead of L2: L2(s-1) is ready-to-run PE work
            # that absorbs any DMA lateness in L1(s)'s inputs.
            emit_l1(0)
            dma_w2(0)
            dma_x(2)
            dma_w1(2)
            emit_l1(1)
            dma_w2(1)
            emit_l2(0)
            dma_x(3)
            dma_w1(3)
            emit_l1(2)
            dma_w2(2)
            emit_l2(1)
            emit_l1(3)
            dma_w2(3)
            emit_l2(2)
            emit_l2(3)

    nc.compile()
    return nc


def build_v3_into(cps, out):
    # thread entrypoint: keeps caller frames (kernel.py, driver) out of the
    # Bass-captured tracebacks so the BIR bytes are fully location-independent
    try:
        out["nc"] = build_v3(cps)
    except BaseException as exc:  # noqa: BLE001
        out["exc"] = exc
'''


def _build_v3(cps):
    """Build via a content-addressed module under /tmp so the generated BIR
    (and hence the neuron compile-cache key) is independent of where this
    file lives."""
    import hashlib
    import importlib.util

    h = hashlib.md5(_BUILDER_SRC.encode()).hexdigest()[:12]
    modname = f"_moe_builder_{h}"
    if modname not in sys.modules:
        path = f"/tmp/_moe_builder_{h}.py"
        try:
            if not (os.path.exists(path)
                    and open(path).read() == _BUILDER_SRC):
                tmp = f"{path}.{os.getpid()}.tmp"
                with open(tmp, "w") as f:
                    f.write(_BUILDER_SRC)
                os.replace(tmp, path)
        except OSError:
            import tempfile

            path = os.path.join(tempfile.mkdtemp(), f"{modname}.py")
            with open(path, "w") as f:
                f.write(_BUILDER_SRC)
        spec = importlib.util.spec_from_file_location(modname, path)
        mod = importlib.util.module_from_spec(spec)
        sys.modules[modname] = mod
        spec.loader.exec_module(mod)
    import threading

    out = {}
    t = threading.Thread(
        target=sys.modules[modname].build_v3_into, args=(cps, out)
    )
    t.start()
    t.join()
    if "exc" in out:
        raise out["exc"]
    return out["nc"]


def _run_with_retry(run_fn, nc, in_maps, tmpdir, attempts=4):
    """Transient NRT/device errors (e.g. NRT_EXEC_UNIT_UNRECOVERABLE right
    after another process released the cores) have been observed; retry with
    growing backoff, resetting the jax backend in between (the failed PJRT
    client state does not recover on its own)."""
    import time

    last_exc = None
    for a in range(attempts):
        try:
            return run_fn(nc, in_maps, core_ids=list(range(E)), tmpdir=tmpdir)
        except Exception as exc:  # noqa: BLE001
            last_exc = exc
            time.sleep(5.0 * (a + 1))
            try:
                import jax

                jax.clear_backends()
            except Exception:
                pass
    raise last_exc


def _pack_inputs(x_flat, idx_per_e, counts, W1, b1, W2, groups, cps):
    """Build the 8 per-core input maps for the quad-split program."""
    CT = sum(cps)
    xbase = [sum(cps[:s]) for s in range(GS)]
    in_maps = [None] * E
    for g in range(NG):
        experts = groups[g]
        # shared-within-group x: per-slot blocks of [P, DT*Cs]
        xt = np.zeros((P, DT * CT), np.float32)
        for s in range(GS):
            e = experts[s]
            cs = cps[s]
            xp_ = np.zeros((cs, D), np.float32)
            xp_[:counts[e]] = x_flat[idx_per_e[e]]
            xt[:, DT * xbase[s]:DT * xbase[s] + DT * cs] = (
                xp_.T.reshape(DT, P, cs).transpose(1, 0, 2)
                .reshape(P, DT * cs)
            )
        xt = np.ascontiguousarray(xt).astype(np.float16)

        for q in range(GS):
            fsl = slice(q * (F // GS), (q + 1) * (F // GS))
            w1c = np.empty((GS, P, FQ * DT * P), np.float16)
            w2c = np.empty((GS, P, DT * FQ * P), np.float16)
            b1c = np.empty((P, GS * FQ), np.float32)
            for s in range(GS):
                e = experts[s]
                # w1c[s][p, (jj*DT+i)*P + c] = W1[e][i*128+p, q*1024+jj*128+c]
                A = W1[e][:, fsl]
                w1c[s] = (
                    A.reshape(DT, P, FQ, P).transpose(1, 2, 0, 3)
                    .reshape(P, FQ * DT * P)
                )
                # w2c[s][p, (m*FQ+j)*P + c] = W2[e][q*1024+j*128+p, m*128+c]
                B = W2[e][fsl, :]
                w2c[s] = (
                    B.reshape(FQ, P, DT, P).transpose(1, 2, 0, 3)
                    .reshape(P, DT * FQ * P)
                )
                # b1c[p, s*FQ+jj] = b1[e][q*1024 + jj*128 + p]
                b1c[:, s * FQ:(s + 1) * FQ] = b1[e][fsl].reshape(FQ, P).T
            in_maps[g * GS + q] = {
                "xt": xt,
                "w1": np.ascontiguousarray(w1c),
                "w2": np.ascontiguousarray(w2c),
                "b1": np.ascontiguousarray(b1c),
            }
    return in_maps


def _emulate_v3(in_maps, cps):
    """Numpy emulation of the device program (layout validation)."""
    results = []
    xbase = [sum(cps[:s]) for s in range(GS)]
    for core in range(E):
        im = in_maps[core]
        xt = im["xt"].astype(np.float32)
        outs = {}
        for s in range(GS):
            cs = cps[s]
            xs = xt[:, DT * xbase[s]:DT * xbase[s] + DT * cs].reshape(
                P, DT, cs
            )
            h = np.zeros((FQ, P, cs), np.float32)
            for jj in range(FQ):
                acc = np.zeros((P, cs), np.float32)
                for i in range(DT):
                    w = im["w1"][s][:, (jj * DT + i) * P:(jj * DT + i + 1) * P]
                    acc += w.astype(np.float32).T @ xs[:, i]
                h[jj] = np.maximum(
                    acc + im["b1"][:, s * FQ + jj][:, None], 0.0
                ).astype(np.float16).astype(np.float32)
            y = np.zeros((P, DT, cs), np.float32)
            for m in range(DT):
                for j in range(FQ):
                    w = im["w2"][s][:, (m * FQ + j) * P:(m * FQ + j + 1) * P]
                    y[:, m] += w.astype(np.float32).T @ h[j]
            outs[f"y{s}"] = y.reshape(P, DT * cs).astype(np.float16)
        results.append(outs)
    return results


def kernel(x, Wg, bg, W1, b1, W2, b2, k):
    global last_results
    emulate = os.environ.get("KERNEL_EMULATE") == "1"
    if not emulate:
        _ensure_ntff_hook()
        from concourse.bass_utils import run_bass_kernel_spmd

    x = np.asarray(x)
    B, S, _ = x.shape
    N = B * S
    x_flat = np.ascontiguousarray(x.reshape(N, D)).astype(np.float32)

    # ---- host router (exact vs fp32 reference; see module docstring) ----
    logits = x_flat.astype(np.float64) @ np.asarray(Wg).astype(np.float64)
    logits += np.asarray(bg).astype(np.float64)
    assign = np.argmax(logits, axis=-1)

    idx_per_e = [np.flatnonzero(assign == e) for e in range(E)]
    counts = [len(ix) for ix in idx_per_e]

    W1 = np.asarray(W1, dtype=np.float32)
    W2 = np.asarray(W2, dtype=np.float32)
    b1 = np.asarray(b1, dtype=np.float32)
    b2 = np.asarray(b2, dtype=np.float32)

    tmpdir = os.environ.get("KERNEL_TMPDIR")

    # Sort experts by count desc; alternate between the two groups so the
    # rank-r experts of both groups have similar counts (rank-matched
    # padding -> minimal SPMD shape padding). Slot 0 is the largest
    # (more early PE work covers the DMA ramp), slot GS-1 the smallest
    # (shortens the drain tail).
    order = list(np.argsort([-c for c in counts], kind="stable"))
    groups = [order[0::2], order[1::2]]
    cps = [
        max(8, -(-max(counts[groups[0][r]], counts[groups[1][r]]) // 8) * 8)
        for r in range(GS)
    ]

    in_maps = _pack_inputs(
        x_flat, idx_per_e, counts, W1, b1, W2, groups, cps
    )

    if emulate:
        results = _emulate_v3(in_maps, cps)
        last_results = None
    else:
        key = ("v3", tuple(cps))
        if key not in _prog_cache:
            _prog_cache[key] = _build_v3(cps)
        nc = _prog_cache[key]
        last_results = _run_with_retry(
            run_bass_kernel_spmd, nc, in_maps, tmpdir
        )
        results = last_results.results

    # ---- gather: sum the GS per-quarter partials, undo the transpose,
    # and concatenate grouped-by-expert (== reference order) ----
    out = np.empty((N, D), np.float32)
    pos = [0] * E
    p = 0
    for e in range(E):
        pos[e] = p
        p += counts[e]
    for g in range(NG):
        for s in range(GS):
            e = groups[g][s]
            cs = cps[s]
            cnt = counts[e]
            acc = np.zeros((P, DT, cs), np.float32)
            for q in range(GS):
                acc += results[g * GS + q][f"y{s}"].reshape(P, DT, cs)
            ye = acc.transpose(1, 0, 2).reshape(D, cs).T[:cnt]
            out[pos[e]:pos[e] + cnt] = ye + b2[e]
    return out.reshape(B, S, D)
